# revision 1
# baseline (speedup 1.0000x reference)
"""Trainium2 Bass kernel for nn_EncoderLayer (dense transformer encoder layer
with static-expansion attention-like block + FF), data-parallel over 8 cores.

Contract: kernel(**inputs) takes FULL unsharded inputs (as in setup_inputs()),
returns the FULL (64, 256, 512) float32 output.

Design (718us baseline -> 377us):
- fp8(e4m3) DoubleRow matmuls for the whole static-expansion block with
  power-of-2 scaling (w6 x32, qgT x16, cfw/b_tab x8) descaled for free in
  the PSUM-read Act/DVE ops; FF stays bf16 (fp8 there breaks accuracy).
- q_tab rows are gathered+transposed on the HOST (idx is an input), so the
  z-scores matmul directly into gathered order: no zfull DRAM roundtrip.
- Software pipeline per batch element with 5 stages: A_pre (loads + LN1
  chain), A_main (PE: x2T/x_keyT/z), B (static expansion), C_pre (LN2
  chain), C_main (PE: x3T/hT/FF2). Emission order [B(b)][A_main(b+1)]
  [C_main(b-1)] keeps every PE-FIFO boundary on work whose small-op
  (DVE/Act) prerequisites were produced a full segment earlier -- the
  in-order engine FIFOs otherwise head-block the PE for ~10us/elem.
- fw/bw normalizers folded into PSUM-read ops (scalar_tensor_tensor with
  per-partition reciprocal scalars); bw denominators via a free-dim
  tensor_reduce over the transposed weights; biases as rank-1 matmuls.
- fp8 PE transposes need output element step 2 (staged via [128,1024,2]).
"""

import sys

for _p in ("/opt/trn_rl_repo",):
    if _p not in sys.path:
        sys.path.insert(0, _p)

import numpy as np
import ml_dtypes

import concourse.bass as bass
import concourse.mybir as mybir
import concourse.tile as tile
from concourse.vector_clock import ScopedClock

F32 = mybir.dt.float32
BF16 = mybir.dt.bfloat16
F8 = mybir.dt.float8e4
I32 = mybir.dt.int32
PM_DR = mybir.MatmulPerfMode.DoubleRow
SCL_W = 32.0   # host scale on w6 + brows (descale 1/32 on PSUM read)
SCL_Q = 16.0   # host scale on qgT (descale 1/16 on z PSUM read)
SCL_C = 8.0    # scale on cfw/b_tab (descale folded into rbw)
AX = mybir.AxisListType
OP = mybir.AluOpType
AF = mybir.ActivationFunctionType

D = 512          # d_model
DFF = 2048       # d_ff
N = 992          # n experts
L = 256          # enc len
BS = 64
NCORES = 8
BPC = BS // NCORES  # batch elements per core
EPS = 1e-9
LN_EPS = 1e-5

KD = D // 128     # 4 k-chunks over d_model
LT = L // 128     # 2 l-chunks
NMO = 8           # n-chunks over N (7x128 + 96)
NSZ = [128] * 7 + [96]
NOFF = [128 * i for i in range(8)]
KF = DFF // 128   # 16 chunks over d_ff

W_K, W_A, W_GA, W_B, W_GB, W_S = range(6)


class SplitDrainTC(tile.TileContext):
    """TileContext whose exit drain splits semaphore waits across nop
    instructions (this walrus build rejects >2 sync waits on one Drain)."""

    def _drain_and_barrier(self, tick_clock, wait_clock):
        nc = self.nc
        probe = nc.sync.nop(nofuse=True)
        wait_clock.add_sem_waits(probe.ins, ScopedClock({None: tick_clock.global_clock}))
        si = probe.ins.sync_info
        waits = list(si.on_wait) if si and si.on_wait else []
        if len(waits) > 1:
            si.on_wait = waits[:1]
            sems_by_name = {h.name: h for h in self.sems.allocated().values()}
            for w in waits[1:]:
                n2 = nc.sync.nop(nofuse=True)
                n2.wait_op(sems_by_name[w.ant_name], w.wait_value, "sem-ge")
        nc.sync.drain()
        nc.all_engine_barrier()
        popped = nc._tile_sem_poison_stack.pop()
        assert popped is self._sem_poison
        nc.clear_and_free_semaphores(list(self.sems.allocated().values()))
        nc.all_engine_barrier()


def _split_excess_waits(nc, cap=2):
    """This walrus build rejects instructions carrying more than ~2 sync
    waits. Hoist excess waits onto same-engine nop instructions inserted
    immediately before the offending instruction (engine program order is
    bb order, so the nop's waits complete first)."""
    import bass_rust
    for f in nc.m.functions:
        for bb in f.blocks:
            over = [inst for inst in bb.instructions
                    if inst.sync_info and inst.sync_info.on_wait
                    and len(inst.sync_info.on_wait) > cap]
            if not over:
                continue
            carriers = {}
            for inst in over:
                waits = list(inst.sync_info.on_wait)
                inst.sync_info.on_wait = waits[:cap]
                rest = waits[cap:]
                lst = []
                for i in range(0, len(rest), cap):
                    nop = nc.engines[inst.engine].nop(nofuse=True)
                    cur = nc.cur_bb.bb
                    assert cur.instructions[-1] is nop.ins
                    cur.instructions.pop()
                    nop.ins.sync_info = bass_rust.SyncInfo(
                        on_wait=rest[i:i + cap], on_update=[])
                    lst.append(nop.ins)
                carriers[inst.name] = lst
            out = []
            for inst in bb.instructions:
                out.extend(carriers.get(inst.name, ()))
                out.append(inst)
            bb.instructions[:] = out


def build_program(n_elems=BPC):
    """Single-core SPMD program; see kernel() for the per-core input map."""
    nc = bass.Bass("TRN2", target_bir_lowering=False, debug=False)

    x_d = nc.dram_tensor("x", [n_elems, L, D], BF16, kind="ExternalInput").ap()
    nidx_d = nc.dram_tensor("nidx", [n_elems, N], I32, kind="ExternalInput").ap()
    mask_d = nc.dram_tensor("mask", [n_elems, N, L], BF16, kind="ExternalInput").ap()
    btab_d = nc.dram_tensor("b_tab", [N, D], F8, kind="ExternalInput").ap()
    qgT_d = nc.dram_tensor("qgT", [n_elems, D, N], F8, kind="ExternalInput").ap()
    w6_d = nc.dram_tensor("w6", [6, D, D], F8, kind="ExternalInput").ap()
    brows_d = nc.dram_tensor("brows", [6 * D], BF16, kind="ExternalInput").ap()
    bkrow_d = nc.dram_tensor("bkrow", [D], F32, kind="ExternalInput").ap()
    wf1_d = nc.dram_tensor("wf1", [D, DFF], BF16, kind="ExternalInput").ap()
    bf1_d = nc.dram_tensor("bf1", [DFF], F32, kind="ExternalInput").ap()
    wf2_d = nc.dram_tensor("wf2", [DFF, D], BF16, kind="ExternalInput").ap()
    bf2_d = nc.dram_tensor("bf2", [D], BF16, kind="ExternalInput").ap()
    out_d = nc.dram_tensor("out", [n_elems, L, D], F32, kind="ExternalOutput").ap()

    with SplitDrainTC(nc) as tc:
        _emit(nc, tc, n_elems, x_d, nidx_d, mask_d, btab_d, qgT_d, w6_d,
              brows_d, bkrow_d, wf1_d, bf1_d, wf2_d, bf2_d, out_d)
    _split_excess_waits(nc, cap=1)
    return nc


def _layer_norm(nc, pool_small, xn, x_sb, eps_tile, tagp):
    """xn[:, lt, :] = (x - mean)/sqrt(var + LN_EPS), bf16 out."""
    for lt in range(LT):
        stats = pool_small.tile([128, 6], F32, tag=tagp + "_st", bufs=2)
        nc.vector.bn_stats(stats[:], x_sb[:, lt, :])
        aggr = pool_small.tile([128, 2], F32, tag=tagp + "_ag", bufs=2)
        nc.vector.bn_aggr(aggr[:], stats[:])
        sv = pool_small.tile([128, 1], F32, tag=tagp + "_sv", bufs=2)
        nc.scalar.activation(sv[:], aggr[:, 1:2], AF.Sqrt, bias=eps_tile[:])
        rstd = pool_small.tile([128, 1], F32, tag=tagp + "_rs", bufs=2)
        nc.vector.reciprocal(rstd[:], sv[:])
        nmr = pool_small.tile([128, 1], F32, tag=tagp + "_nm", bufs=2)
        nc.vector.tensor_scalar(out=nmr[:], in0=aggr[:, 0:1], scalar1=rstd[:],
                                scalar2=-1.0, op0=OP.mult, op1=OP.mult)
        nc.scalar.activation(xn[:, lt, :], x_sb[:, lt, :], AF.Identity,
                             bias=nmr[:], scale=rstd[:])


class Ctx:
    pass


def _emit(nc, tc, n_elems, x_d, nidx_d, mask_d, btab_d, qgT_d, w6_d, brows_d,
          bkrow_d, wf1_d, bf1_d, wf2_d, bf2_d, out_d):
    from contextlib import ExitStack

    c = Ctx()
    c.n_elems = n_elems
    c.x_d, c.nidx_d, c.mask_d, c.btab_d, c.out_d = x_d, nidx_d, mask_d, btab_d, out_d
    c.qgT_d = qgT_d

    top = ExitStack()
    with top:
        # ---- persistent constants ----
        const_pool = top.enter_context(tc.tile_pool(name="const", bufs=1))
        ident = const_pool.tile([128, 128], F32)
        from concourse.masks import make_identity
        make_identity(nc, ident[:])
        identb = const_pool.tile([128, 128], BF16)
        nc.vector.tensor_copy(identb[:], ident[:])
        c.identb = identb
        ident8 = const_pool.tile([128, 128], F8)
        nc.vector.tensor_copy(ident8[:], ident[:])
        c.ident8 = ident8
        ones_row = const_pool.tile([1, 128], BF16)
        nc.vector.memset(ones_row[:], 1.0)
        c.ones_row = ones_row
        eps_tile = const_pool.tile([128, 1], F32)
        nc.vector.memset(eps_tile[:], LN_EPS)
        c.eps_tile = eps_tile
        bk_col = const_pool.tile([128, KD], F32)
        nc.sync.dma_start(bk_col[:], bkrow_d.rearrange("(k p) -> p k", p=128))
        c.bk_col = bk_col
        bf1_col = const_pool.tile([128, KF], F32)
        nc.sync.dma_start(bf1_col[:], bf1_d.rearrange("(k p) -> p k", p=128))
        c.bf1_col = bf1_col

        # ---- PSUM pools ----
        c.ps_mm = top.enter_context(tc.tile_pool(name="ps_mm", bufs=3, space="PSUM"))
        c.ps_h = top.enter_context(tc.tile_pool(name="ps_h", bufs=3, space="PSUM"))
        c.ps_tr = top.enter_context(tc.tile_pool(name="ps_tr", bufs=2, space="PSUM"))

        c.small = top.enter_context(tc.tile_pool(name="small", bufs=2))

        # ---- weights (persistent; w6/qgT/b_tab fp8, FF bf16) ----
        wpool = top.enter_context(tc.tile_pool(name="w", bufs=1))
        w6_sb = wpool.tile([128, 6, KD, D], F8)
        for wi in range(6):
            nc.gpsimd.dma_start(w6_sb[:, wi, :, :],
                                w6_d[wi].rearrange("(k p) n -> p k n", p=128))
        c.w6_sb = w6_sb

        # bias rows (x SCL_W on host) for rank-1 bias matmuls
        brow_sb = const_pool.tile([1, 6 * D], BF16)
        nc.sync.dma_start(brow_sb[:], brows_d.rearrange("(o a) -> o a", o=1))
        c.brow_sb = brow_sb
        bf2_st = const_pool.tile([1, D], BF16)
        nc.sync.dma_start(bf2_st[:], bf2_d.rearrange("(o a) -> o a", o=1))
        c.bf2_st = bf2_st

        wf1_sb = wpool.tile([128, KD, DFF], BF16)
        nc.gpsimd.dma_start(wf1_sb[:, :, :DFF // 2],
                            wf1_d[:, :DFF // 2].rearrange("(k p) n -> p k n", p=128))
        nc.gpsimd.dma_start(wf1_sb[:, :, DFF // 2:],
                            wf1_d[:, DFF // 2:].rearrange("(k p) n -> p k n", p=128))
        wf2_sb = wpool.tile([128, KF, D], BF16)
        nc.gpsimd.dma_start(wf2_sb[:], wf2_d.rearrange("(k p) n -> p k n", p=128))
        c.wf1_sb, c.wf2_sb = wf1_sb, wf2_sb

        # ---- per-elem pools ----
        c.A = top.enter_context(tc.tile_pool(name="A", bufs=2))
        c.B = top.enter_context(tc.tile_pool(name="B", bufs=1))
        c.C = top.enter_context(tc.tile_pool(name="C", bufs=1))
        c.Y = top.enter_context(tc.tile_pool(name="Y", bufs=2))

        c.pre_st = {}  # per-elem A_pre tiles passed to A_main
        c.a_st = {}    # per-elem stage-A tiles passed to B
        c.y_st = {}    # per-elem y2 tiles passed to C
        c.c_st = {}    # per-elem x3 tiles passed from C_pre to C_main

        # ---- software pipeline ----
        # Per-elem stages: A_pre (x/idx loads + LN1 chain), A_main (PE: x2T,
        # x_keyT, z), B (static expansion), C_pre (LN2 chain), C_main (PE:
        # x3T, hT, FF2). Emission order keeps every PE-FIFO boundary on work
        # whose small-op (DVE/Act) prerequisites were produced a full segment
        # earlier:  [B(b)][A_main(b+1)][C_main(b-1)] on the PE, with
        # C_pre(b)/A_pre(b+2) small ops slotted right after B(b)'s DVE tail.
        _stage_a_pre(nc, c, 0)
        if n_elems > 1:
            _stage_a_pre(nc, c, 1)
        _stage_a_main(nc, c, 0)
        for b in range(n_elems):
            _stage_b(nc, c, b)
            _stage_c_pre(nc, c, b)
            if b + 2 < n_elems:
                _stage_a_pre(nc, c, b + 2)
            if b + 1 < n_elems:
                _stage_a_main(nc, c, b + 1)
            if b >= 1:
                _stage_c_main(nc, c, b - 1)
        _stage_c_main(nc, c, n_elems - 1)


def _stage_a_pre(nc, c, b):
    """Load x/idx + LN1 chain (small DVE/Act ops, emitted early)."""
    A, small = c.A, c.small
    x_sb = A.tile([128, LT, D], BF16, tag="x", bufs=3)
    nc.sync.dma_start(x_sb[:], c.x_d[b].rearrange("(lt p) d -> p lt d", p=128))
    idx_sb = A.tile([128, NMO], I32, tag="idx", bufs=3)
    nc.sync.dma_start(idx_sb[:, 0:7],
                      c.nidx_d[b, 0:896].rearrange("(a p) -> p a", p=128))
    nc.sync.dma_start(idx_sb[0:96, 7:8],
                      c.nidx_d[b, 896:992].rearrange("(a p) -> p a", p=96))
    # LN1
    xn = A.tile([128, LT, D], BF16, tag="xn", bufs=3)
    _layer_norm(nc, small, xn, x_sb, c.eps_tile, "ln1")
    c.pre_st[b] = (x_sb, idx_sb, xn)


def _stage_a_main(nc, c, b):
    """PE work: x2T, x_keyT, z = qgT^T @ x_keyT; qgT/mask DMAs + bexp gather."""
    A, small = c.A, c.small
    x_sb, idx_sb, xn = c.pre_st.pop(b)
    qgT_sb = A.tile([128, KD, N], F8, tag="qgT")
    nc.sync.dma_start(qgT_sb[:], c.qgT_d[b].rearrange("(k p) n -> p k n", p=128))
    maskf = A.tile([128, NMO, L], BF16, tag="maskf")
    nc.sync.dma_start(maskf[:, 0:7, :],
                      c.mask_d[b, 0:896, :].rearrange("(a p) l -> p a l", p=128))
    nc.sync.dma_start(maskf[0:96, 7, :], c.mask_d[b, 896:992, :])

    # bias_exp gather (gpsimd queue; consumed mid-B; rows pre-scaled x SCL_C)
    bexp = A.tile([128, NMO, D], F8, tag="bexp")
    for mo in range(NMO):
        m = NSZ[mo]
        nc.gpsimd.indirect_dma_start(
            out=bexp[:m, mo, :], out_offset=None, in_=c.btab_d[:, :],
            in_offset=bass.IndirectOffsetOnAxis(ap=idx_sb[:m, mo:mo + 1], axis=0))

    # x2T: [128(d%128), KD, L] fp8, via PE transposes (bf16 psum, fp8 copy-out)
    x2T = A.tile([128, KD, L], F8, tag="x2T")
    for kp in range(KD // 2):
        ps = c.ps_tr.tile([128, 2048], F8, tag="tr")
        psb = ps[:].bitcast(BF16)
        for k2 in range(2):
            ko = kp * 2 + k2
            for lt in range(LT):
                off = (k2 * LT + lt) * 128
                nc.tensor.transpose(psb[:, off:off + 128],
                                    xn[:, lt, ko * 128:(ko + 1) * 128], c.identb[:])
        nc.scalar.copy(x2T[:, kp * 2:kp * 2 + 2, :], psb[:, 0:512])

    # x_keyT = (xn @ Wk')^T + bk' : [128, KD, L] fp8 (DoubleRow, descale 1/32)
    xkT = A.tile([128, KD, L], F8, tag="xkT")
    for ko in range(KD):
        ps = c.ps_h.tile([128, L], F32, tag="h")
        for kp in range(KD // 2):
            nc.tensor.matmul(ps[:], c.w6_sb[:, W_K, 2 * kp:2 * kp + 2,
                                            ko * 128:(ko + 1) * 128],
                             x2T[:, 2 * kp:2 * kp + 2, :], start=(kp == 0),
                             stop=(kp == KD // 2 - 1), perf_mode=PM_DR)
        nc.scalar.activation(xkT[:, ko, :], ps[:], AF.Identity,
                             bias=c.bk_col[:, ko:ko + 1], scale=1.0 / SCL_W)

    # z[n, l] = qgT[:, n]^T x_keyT[:, l]  (DoubleRow, descale 1/16)
    z_sb = A.tile([128, NMO, L], BF16, tag="z")
    for mo in range(NMO):
        m = NSZ[mo]
        ps = c.ps_h.tile([128, L], F32, tag="h")
        for kp in range(KD // 2):
            nc.tensor.matmul(ps[:m, :],
                             qgT_sb[:, 2 * kp:2 * kp + 2, NOFF[mo]:NOFF[mo] + m],
                             xkT[:, 2 * kp:2 * kp + 2, :], start=(kp == 0),
                             stop=(kp == KD // 2 - 1), perf_mode=PM_DR)
        nc.scalar.activation(z_sb[:m, mo, :], ps[:m, :], AF.Copy,
                             scale=1.0 / SCL_Q)

    c.a_st[b] = (x_sb, x2T, maskf, bexp, z_sb)


def _stage_b(nc, c, b):
    """Static expansion for elem b; writes y2 (SBUF, bf16) for stage C."""
    x_sb, x2T, maskf, bexp, z_sb = c.a_st.pop(b)
    B, small = c.B, c.small

    # az = relu(z)*m; bz = min(z,0)*m (fp8); row sums -> fw normalizers
    az = B.tile([128, NMO, L], F8, tag="az")
    bz = B.tile([128, NMO, L], F8, tag="bz")
    sum_a = small.tile([128, NMO], F32, tag="sum_a")
    sum_b = small.tile([128, NMO], F32, tag="sum_b")
    nc.vector.memset(sum_a[:], 1.0)
    nc.vector.memset(sum_b[:], 1.0)
    # zero the mo=7 tail rows so DoubleRow mo-pairs see 0 contributions
    nc.vector.memset(az[96:128, 7, :], 0.0)
    nc.vector.memset(bz[96:128, 7, :], 0.0)
    for mo in range(NMO):
        m = NSZ[mo]
        nc.vector.scalar_tensor_tensor(
            out=az[:m, mo, :], in0=z_sb[:m, mo, :], scalar=0.0,
            in1=maskf[:m, mo, :], op0=OP.max, op1=OP.mult,
            accum_out=sum_a[:m, mo:mo + 1])
        nc.vector.scalar_tensor_tensor(
            out=bz[:m, mo, :], in0=z_sb[:m, mo, :], scalar=0.0,
            in1=maskf[:m, mo, :], op0=OP.min, op1=OP.mult,
            accum_out=sum_b[:m, mo:mo + 1])
    # rfw = SCL_C / (sum +- eps)  (cfw written x SCL_C for fp8 range)
    rfw_a = small.tile([128, NMO], F32, tag="rfw_a")
    rfw_b = small.tile([128, NMO], F32, tag="rfw_b")
    tmp_a = small.tile([128, NMO], F32, tag="tmp_a")
    tmp_b = small.tile([128, NMO], F32, tag="tmp_b")
    nc.vector.tensor_scalar(out=tmp_a[:], in0=sum_a[:], scalar1=1.0 / SCL_C,
                            scalar2=EPS / SCL_C, op0=OP.mult, op1=OP.add)
    nc.vector.reciprocal(rfw_a[:], tmp_a[:])
    nc.vector.tensor_scalar(out=tmp_b[:], in0=sum_b[:], scalar1=1.0 / SCL_C,
                            scalar2=-EPS / SCL_C, op0=OP.mult, op1=OP.add)
    nc.vector.reciprocal(rfw_b[:], tmp_b[:])

    # gated embeddings + sel gate (DoubleRow + rank-1 bias matmul; the Act
    # read descales by 1/SCL_W -- bias rows are host-scaled by SCL_W)
    def mm_dr_bias(ps, wi, lt, brow):
        for kp in range(KD // 2):
            nc.tensor.matmul(ps[:], x2T[:, 2 * kp:2 * kp + 2,
                                        lt * 128:(lt + 1) * 128],
                             c.w6_sb[:, wi, 2 * kp:2 * kp + 2, :],
                             start=(kp == 0), stop=False, perf_mode=PM_DR)
        nc.tensor.matmul(ps[:], c.ones_row[:], brow, start=False, stop=True)

    def gated_emb(wi, wgi, tag):
        emb = B.tile([128, LT, D], F8, tag=tag)
        for lt in range(LT):
            ps_g = c.ps_mm.tile([128, D], F32, tag="mm")
            mm_dr_bias(ps_g, wgi, lt, c.brow_sb[:, wgi * D:(wgi + 1) * D])
            sig = B.tile([128, D], BF16, tag="sig", bufs=2)
            nc.scalar.activation(sig[:], ps_g[:], AF.Sigmoid, scale=1.0 / SCL_W)
            ps_a = c.ps_mm.tile([128, D], F32, tag="mm")
            mm_dr_bias(ps_a, wi, lt, c.brow_sb[:, wi * D:(wi + 1) * D])
            nc.vector.scalar_tensor_tensor(
                out=emb[:, lt, :], in0=ps_a[:], scalar=1.0 / SCL_W,
                in1=sig[:], op0=OP.mult, op1=OP.mult)
        return emb

    a_emb = gated_emb(W_A, W_GA, "a_emb")
    b_emb = gated_emb(W_B, W_GB, "b_emb")

    sel = B.tile([128, LT, D], BF16, tag="sel")
    for lt in range(LT):
        ps_s = c.ps_mm.tile([128, D], F32, tag="mm")
        mm_dr_bias(ps_s, W_S, lt, c.brow_sb[:, W_S * D:(W_S + 1) * D])
        nc.scalar.activation(sel[:, lt, :], ps_s[:], AF.Sigmoid,
                             scale=1.0 / SCL_W)

    # fw/bw, both sides interleaved phase-by-phase so the cfw DVE stts always
    # have PE matmul cover: [fwT a,b] [cfw a,b] [bw a,b]
    y2 = c.Y.tile([128, LT, D], F32, tag="y2")
    out_a = B.tile([128, LT, D], BF16, tag="out_a")
    zzs = [az, bz]
    rfws = [rfw_a, rfw_b]
    embs = [a_emb, b_emb]
    fwTs, rbws, cfws = [], [], []

    for side in range(2):
        zz = zzs[side]
        # fwT[l, n] = zz[n, l]^T  (fp8; HW fp8 transpose writes elem-step-2,
        # so stage through a [128, 1024, 2] view and read the even bytes)
        fwT = B.tile([128, LT, N], F8, tag="fwT", bufs=2)
        ps_lt = [c.ps_tr.tile([128, 1024, 2], F8, tag="tr",
                              name=f"trf_{b}_{side}_{lt}")
                 for lt in range(LT)]
        for mo in range(NMO):
            m = NSZ[mo]
            for lt in range(LT):
                nc.tensor.transpose(ps_lt[lt][:, NOFF[mo]:NOFF[mo] + m, 0:1],
                                    zz[:m, mo, lt * 128:(lt + 1) * 128],
                                    c.ident8[:m, :m])
        for lt in range(LT):
            nc.scalar.copy(fwT[:, lt, :], ps_lt[lt][:, 0:N, 0:1])

        # bw denominator: den[l] = sum_n zz[n, l] (free-dim reduce over fwT)
        # rbw folds the 1/SCL_C descale of the fp8 cfw values.
        den = small.tile([128, LT], F32, tag="den", bufs=2)
        nc.vector.tensor_reduce(out=den[:], in_=fwT[:], axis=AX.X, op=OP.add)
        rbw = small.tile([128, LT], F32, tag="rbw", bufs=2)
        tmp2 = small.tile([128, LT], F32, tag="tmp2", bufs=2)
        nc.vector.tensor_scalar(out=tmp2[:], in0=den[:], scalar1=SCL_C,
                                scalar2=EPS * SCL_C if side == 0 else -EPS * SCL_C,
                                op0=OP.mult, op1=OP.add)
        nc.vector.reciprocal(rbw[:], tmp2[:])
        fwTs.append(fwT)
        rbws.append(rbw)

    for side in range(2):
        # fw matmul (DoubleRow over the two l-chunks):
        # cfw[n, d] = SCL_C * (rfw_true[n] * sum_l fwT[l, n]^T emb[l, d] + bexp)
        cfw = B.tile([128, NMO, D], F8, tag="cfw", bufs=2)
        # zero mo=7 tail rows (never stt-written; DoubleRow pair reads them)
        nc.vector.memset(cfw[96:128, 7, :], 0.0)
        for mo in range(NMO):
            m = NSZ[mo]
            ps = c.ps_mm.tile([128, D], F32, tag="mm")
            nc.tensor.matmul(ps[:m, :], fwTs[side][:, 0:2, NOFF[mo]:NOFF[mo] + m],
                             embs[side][:, 0:2, :], start=True, stop=True,
                             perf_mode=PM_DR)
            nc.vector.scalar_tensor_tensor(
                out=cfw[:m, mo, :], in0=ps[:m, :],
                scalar=rfws[side][:m, mo:mo + 1],
                in1=bexp[:m, mo, :], op0=OP.mult, op1=OP.add)
        cfws.append(cfw)

    for side in range(2):
        zz = zzs[side]
        rbw = rbws[side]
        cfw = cfws[side]
        # bw matmul (DoubleRow over mo-pairs; az/bz mo=7 tail rows are zeroed):
        # out[l, d] = rbw[l] * sum_n zz[n, l] cfw[n, d]
        for lt in range(LT):
            ps = c.ps_mm.tile([128, D], F32, tag="mm")
            for mp in range(NMO // 2):
                nc.tensor.matmul(ps[:], zz[:, 2 * mp:2 * mp + 2,
                                           lt * 128:(lt + 1) * 128],
                                 cfw[:, 2 * mp:2 * mp + 2, :],
                                 start=(mp == 0), stop=(mp == NMO // 2 - 1),
                                 perf_mode=PM_DR)
            if side == 0:
                nc.scalar.activation(out_a[:, lt, :], ps[:], AF.Copy,
                                     scale=rbw[:, lt:lt + 1])
            else:
                # y2 = (x + out_b) - sel*(out_b - out_a)
                ob = B.tile([128, D], F32, tag="ob", bufs=1)
                nc.vector.scalar_tensor_tensor(
                    out=ob[:], in0=ps[:], scalar=rbw[:, lt:lt + 1],
                    in1=x_sb[:, lt, :], op0=OP.mult, op1=OP.add)
                dt_ = B.tile([128, D], BF16, tag="dt", bufs=1)
                nc.vector.scalar_tensor_tensor(
                    out=dt_[:], in0=ps[:], scalar=rbw[:, lt:lt + 1],
                    in1=out_a[:, lt, :], op0=OP.mult, op1=OP.subtract)
                mt = B.tile([128, D], BF16, tag="mt", bufs=1)
                nc.vector.tensor_tensor(out=mt[:], in0=dt_[:],
                                        in1=sel[:, lt, :], op=OP.mult)
                nc.vector.tensor_tensor(out=y2[:, lt, :], in0=ob[:], in1=mt[:],
                                        op=OP.subtract)
    c.y_st[b] = y2


def _stage_c_pre(nc, c, b):
    """LN2 chain for elem b (small DVE/Act ops, emitted right after B(b))."""
    C, small = c.C, c.small
    y2 = c.y_st[b]
    x3 = C.tile([128, LT, D], BF16, tag="x3", bufs=2)
    _layer_norm(nc, small, x3, y2, c.eps_tile, "ln2")
    c.c_st[b] = x3


def _stage_c_main(nc, c, b):
    """Feed-forward for elem b: out = y2 + relu(LN2(y2) @ Wf1 + bf1) @ Wf2 + bf2."""
    y2 = c.y_st.pop(b)
    x3 = c.c_st.pop(b)
    C, small = c.C, c.small
    x3T = C.tile([128, KD, L], BF16, tag="x3T")
    for kp in range(KD // 2):
        ps = c.ps_tr.tile([128, 1024], BF16, tag="tr")
        for k2 in range(2):
            ko = kp * 2 + k2
            for lt in range(LT):
                off = (k2 * LT + lt) * 128
                nc.tensor.transpose(ps[:, off:off + 128],
                                    x3[:, lt, ko * 128:(ko + 1) * 128], c.identb[:])
        nc.scalar.copy(x3T[:, kp * 2:kp * 2 + 2, :], ps[:, 0:512])

    hT = C.tile([128, KF, L], BF16, tag="hT")
    for mo in range(KF):
        ps = c.ps_h.tile([128, L], F32, tag="h")
        for ki in range(KD):
            nc.tensor.matmul(ps[:], c.wf1_sb[:, ki, mo * 128:(mo + 1) * 128],
                             x3T[:, ki, :], start=(ki == 0), stop=(ki == KD - 1))
        nc.scalar.activation(hT[:, mo, :], ps[:], AF.Relu,
                             bias=c.bf1_col[:, mo:mo + 1])

    out_sb = C.tile([128, LT, D], F32, tag="out_sb", bufs=2)
    for lt in range(LT):
        ps = c.ps_mm.tile([128, D], F32, tag="mm")
        for mo in range(KF):
            nc.tensor.matmul(ps[:], hT[:, mo, lt * 128:(lt + 1) * 128],
                             c.wf2_sb[:, mo, :], start=(mo == 0), stop=False)
        nc.tensor.matmul(ps[:], c.ones_row[:], c.bf2_st[:], start=False,
                         stop=True)
        nc.vector.tensor_tensor(out=out_sb[:, lt, :], in0=ps[:],
                                in1=y2[:, lt, :], op=OP.add)
    nc.sync.dma_start(c.out_d[b].rearrange("(lt p) d -> p lt d", p=128), out_sb[:])


# ---------------------------------------------------------------------------
# host-side weight preprocessing + SPMD launch
# ---------------------------------------------------------------------------

BF_NP = ml_dtypes.bfloat16
F8_NP = ml_dtypes.float8_e4m3fn


def _prep_host(inputs):
    f = lambda k: np.ascontiguousarray(np.asarray(inputs[k], dtype=np.float32))
    g1, b1 = f("ln1_g"), f("ln1_b")
    g2, b2 = f("ln2_g"), f("ln2_b")
    Wk, bk = f("Wk"), f("bk")
    Wa, ba = f("Wa"), f("ba")
    Wa1, ba1 = f("Wa1"), f("ba1")
    Wb, bb = f("Wb"), f("bb")
    Wb1, bb1 = f("Wb1"), f("bb1")
    Ws, bsel = f("Ws"), f("bsel")
    Wf1, bf1 = f("Wf1"), f("bf1")
    Wf2, bf2 = f("Wf2"), f("bf2")
    q_tab, b_tab = f("q_tab"), f("b_tab")

    Waa1 = Wa @ Wa1
    Wbb1 = Wb @ Wb1
    w6 = np.stack([
        g1[:, None] * Wk,
        g1[:, None] * Wa,
        g1[:, None] * Waa1,
        g1[:, None] * Wb,
        g1[:, None] * Wbb1,
        g1[:, None] * Ws,
    ]).astype(np.float32)
    brows = np.stack([
        b1 @ Wk + bk,
        b1 @ Wa + ba,
        b1 @ Waa1 + ba @ Wa1 + ba1,
        b1 @ Wb + bb,
        b1 @ Wbb1 + bb @ Wb1 + bb1,
        b1 @ Ws + bsel,
    ]).astype(np.float32)
    wf1 = np.ascontiguousarray(g2[:, None] * Wf1)
    bf1p = (b2 @ Wf1 + bf1).astype(np.float32)
    bf = lambda a: np.ascontiguousarray(a.astype(BF_NP))
    f8 = lambda a, s: np.ascontiguousarray((a * s).astype(F8_NP))
    # w6/brows x32 (descaled 1/32 on PSUM read; bk applied post-descale),
    # b_tab x8 (cfw fp8 scale, descaled via rbw)
    return dict(b_tab=f8(b_tab, 8.0), w6=f8(w6, 32.0),
                brows=bf(brows.reshape(-1) * 32.0),
                bkrow=np.ascontiguousarray(brows[0].astype(np.float32)),
                wf1=bf(wf1), bf1=np.ascontiguousarray(bf1p),
                wf2=bf(Wf2), bf2=bf(bf2))


_NC_CACHE = {}


def _get_program(n_elems=BPC):
    if n_elems not in _NC_CACHE:
        _NC_CACHE[n_elems] = build_program(n_elems)
    return _NC_CACHE[n_elems]


def make_in_maps(inputs):
    x = np.asarray(inputs["x"], dtype=np.float32).astype(BF_NP)
    nidx = np.asarray(inputs["n_indexes"]).astype(np.int32)
    mask = np.asarray(inputs["mask"]).astype(BF_NP)
    # host-side gather of the scaled q rows, pre-transposed: qgT[e, d, n]
    q_tab = np.asarray(inputs["q_tab"], dtype=np.float32)
    qt_s = (q_tab * (16.0 / np.sqrt(np.float32(D)))).astype(F8_NP)
    qgT = np.ascontiguousarray(np.swapaxes(qt_s[nidx], 1, 2))  # (BS, D, N) fp8 x16
    shared = _prep_host(inputs)
    in_maps = []
    for c in range(NCORES):
        sl = slice(c * BPC, (c + 1) * BPC)
        in_maps.append({
            "x": np.ascontiguousarray(x[sl]),
            "nidx": np.ascontiguousarray(nidx[sl]),
            "mask": np.ascontiguousarray(mask[sl]),
            "qgT": np.ascontiguousarray(qgT[sl]),
            **shared,
        })
    return in_maps


def kernel(**inputs):
    from concourse.bass_utils import run_bass_kernel_spmd

    nc = _get_program(BPC)
    in_maps = make_in_maps(inputs)
    res = run_bass_kernel_spmd(nc, in_maps, core_ids=list(range(NCORES)))
    out = np.concatenate([res.results[c]["out"] for c in range(NCORES)], axis=0)
    return out.astype(np.float32)

